# revision 4
# baseline (speedup 1.0000x reference)
"""Multi-head attention TRN2 Bass kernel, v1 (transposed-AV + multi-engine exp).

Problem: B=8, S=1024, D=768, H=12 heads of DH=64 (torch-style per-head
Linear Q/K/V, softmax over keys, attn @ V, heads concatenated).

Sharding: data-parallel over batch - one batch element per NeuronCore.

Per-core strategy (cost-model-driven redesign of the v0 baseline):
  - Q/K projections in fp32r (score precision is the error budget's
    dominant term: out rel err ~= 2x the score perturbation).
  - Scores computed transposed, scoresT[t,s] = kt.T @ qt per head, fp32r,
    [128t, 1024s] psum groups (2 banks x 2 bufs).
  - exp is split across engines: the scalar engine does ~58% directly
    from PSUM (Exp, scale=1/8, fp16 out); the rest are prescaled s/8 and
    copied PSUM->SBUF fp16 (DVE tensor_scalar_mul or scalar-engine Copy),
    then the *gpsimd* engine computes e**y via the pow ALU op with a
    broadcast e-constant (exact to fp16 rounding; gpsimd has no PSUM port,
    hence the staging copy).
  - AV runs in the output-natural orientation: stationary = 128x128 slices
    of the fp16 exp tiles, moving = [V_h | 1] 65-wide fp16 slots
    (cost-model matmul charge = out free size, so this halves AV cost vs
    the transposed orientation and eliminates the transpose-back pass).
    The ones column accumulates the softmax denominator per (s-row, head).
  - V projection in fp16 from a separate fp16 copy of x; the V bias is
    folded into V itself (softmax weights sum to 1, so out = attn@(V+bv)
    = attn@V + bv exactly), making the post-pass a single DVE divide with
    a broadcast denominator per (pair, s-block).
  - Output assembled per 128-row s-block in SBUF, DMA'd as contiguous
    [128, 768] rows.
"""

import numpy as np
import ml_dtypes

import concourse.bass as bass
import concourse.mybir as mybir
import concourse.tile as tile
from concourse import bacc
from concourse import bass_utils

H, DH = 12, 64
B, S, D = 8, 1024, 768
NPAIR = H // 2
NCORES = 8
NT = S // 128            # t-chunks per head (8)
SB = S // 128            # s-blocks (8; 4 per s-half)
SHW = 512                # s-half width

F32 = mybir.dt.float32
F32R = mybir.dt.float32r
F16 = mybir.dt.float16
AF = mybir.ActivationFunctionType
ALU = mybir.AluOpType

import os
EXP_E = float(np.float32(np.exp(1.0)))  # pow base; scores prescaled by 1/8

# exp-tile engine assignment: of 96 tiles, NPOOL go to gpsimd-pow
NPOOL = 24
SC_BUFS = 3
SM_BUFS = 2
POST_FIRST = 1
UNIPSUM = 0
# every 5th pool tile staged by the scalar engine instead of DVE
ACT_COPY_SLOTS = {9}


def _emit(ctx, tc, nc, xT, xTb, wqk, wvb, bqk, bvf, out):
    P = 128
    const = ctx.enter_context(tc.tile_pool(name="const", bufs=1))
    xpool = ctx.enter_context(tc.tile_pool(name="xpool", bufs=3))
    xbpool = ctx.enter_context(tc.tile_pool(name="xbpool", bufs=3))
    qkpool = ctx.enter_context(tc.tile_pool(name="qkpool", bufs=1))
    vpool = ctx.enter_context(tc.tile_pool(name="vpool", bufs=1))
    stg = ctx.enter_context(tc.tile_pool(name="stg", bufs=6))
    expp = ctx.enter_context(tc.tile_pool(name="expp", bufs=38))
    opool = ctx.enter_context(tc.tile_pool(name="opool", bufs=8))
    rcp = ctx.enter_context(tc.tile_pool(name="rcp", bufs=4))
    psum = ctx.enter_context(tc.tile_pool(name="psum", bufs=1, space="PSUM"))

    # ---- constants (DMA order: needed-first) ----
    wqk_t = const.tile([P, NPAIR * 256], F32R, tag="wqk")
    bias_t = const.tile([P, 2 * NPAIR], F32, tag="bqk")
    wv_t = const.tile([P, NPAIR * 128], F16, tag="wv")
    bvf_t = const.tile([P, D], F32, tag="bvf")
    ec = const.tile([P, 1], F32, tag="ec")
    nc.vector.memset(ec[:], EXP_E)
    nc.sync.dma_start(out=wqk_t[:, 0:256], in_=wqk[:, 0:256])

    # ---- PE p-state warmup: ~3us of junk matmuls on a memset const tile
    # during the initial DMA wait, so the real stream starts at full clock
    wmc = const.tile([P, SHW], F16, tag="wmc")
    nc.vector.memset(wmc[:], 0.0)
    wps = psum.tile([P, SHW], F32, tag="sm", bufs=SM_BUFS, name="wm")
    for _ in range(8):
        nc.tensor.matmul(
            wps[:], wmc[:, 0:P], wmc[:], start=True, stop=True
        )

    # ---- x tiles ----
    xt, xtb = [], []
    did_consts = [False]

    def emit_x():
        for p in range(NPAIR):
            t = xpool.tile([P, S], F32R, tag="x", name=f"x{p}")
            for hh in range(2):
                nc.sync.dma_start(
                    out=t[:, SHW * hh : SHW * (hh + 1)],
                    in_=xT[P * p : P * (p + 1), SHW * hh : SHW * (hh + 1)],
                )
            xt.append(t)
            if p == 0:
                nc.sync.dma_start(out=bias_t[:], in_=bqk[:])
            tb = xbpool.tile([P, S], F16, tag="xb", name=f"xb{p}")
            nc.sync.dma_start(out=tb[:], in_=xTb[P * p : P * (p + 1), :])
            xtb.append(tb)
            if not did_consts[0] and p == 0:
                nc.sync.dma_start(out=wv_t[:], in_=wvb[:])
            if not did_consts[0] and p + 1 < NPAIR:
                nc.sync.dma_start(
                    out=wqk_t[:, 256 * (p + 1) : 256 * (p + 2)],
                    in_=wqk[:, 256 * (p + 1) : 256 * (p + 2)],
                )
        if not did_consts[0]:
            nc.sync.dma_start(out=bvf_t[:], in_=bvf[:])
            did_consts[0] = True

    # ---- Q/K projections ----
    QT, KT, VS = [], [], []

    def emit_qk(p):
        qt = qkpool.tile([P, S], F32R, tag=f"q{p}", name=f"q{p}")
        kt = qkpool.tile([P, S], F32R, tag=f"k{p}", name=f"k{p}")
        for which, dst in ((0, qt), (1, kt)):
            wcol = 2 * p + which
            for sh in range(2):
                ps = psum.tile([P, SHW], F32, tag="sm", bufs=SM_BUFS, name="pj")
                nc.tensor.matmul(
                    ps[:],
                    wqk_t[:, wcol * P : (wcol + 1) * P],
                    xt[p][:, SHW * sh : SHW * (sh + 1)],
                    start=True,
                    stop=True,
                )
                # DVE only: the scalar engine writing f32r tiles yields NaN
                nc.vector.tensor_scalar_add(
                    dst[:, SHW * sh : SHW * (sh + 1)],
                    ps[:],
                    bias_t[:, wcol : wcol + 1],
                )
        QT.append(qt)
        KT.append(kt)

    # ---- V projection (fp16, bias folded into V) ----
    def emit_v(p):
        vs = vpool.tile([P, NT * 130], F16, tag=f"v{p}", name=f"v{p}")
        # denominator ones columns at col 64 of each 65-wide head slot
        ones = vs[:].rearrange("p (t h j) -> p t h j", t=NT, h=2, j=65)
        nc.vector.memset(ones[:, :, :, 64:65], 1.0)
        bvfb = (
            bvf_t[:, P * p : P * (p + 1)]
            .rearrange("p (h j) -> p h j", h=2, j=64)
            .unsqueeze(1)
            .broadcast_to([P, 4, 2, 64])
        )
        for vh in range(2):
            pv = psum.tile([P, 512], F32, tag="sm", bufs=SM_BUFS, name="pv")
            for c in range(4):
                tau = 4 * vh + c
                nc.tensor.matmul(
                    pv[:, P * c : P * (c + 1)],
                    xtb[p][:, P * tau : P * (tau + 1)],
                    wv_t[:, P * p : P * (p + 1)],
                    start=True,
                    stop=True,
                )
            src = pv[:].rearrange("p (c h j) -> p c h j", c=4, h=2, j=64)
            dst = vs[:, 520 * vh : 520 * (vh + 1)].rearrange(
                "p (c h j) -> p c h j", c=4, h=2, j=65
            )[:, :, :, 0:64]
            nc.vector.tensor_tensor(dst, src, bvfb, op=ALU.add)
        VS.append(vs)

    # ---- scores + exp ----
    exp_idx = [0]
    pool_idx = [0]

    def emit_exp(et, pg):
        i = exp_idx[0]
        exp_idx[0] += 1
        if i < 88 and (i * NPOOL) // 88 != ((i + 1) * NPOOL) // 88:
            st = stg.tile([P, 1024], F16, tag="st", name="st")
            j = pool_idx[0]
            pool_idx[0] += 1
            if j % 7 in ACT_COPY_SLOTS:
                nc.scalar.activation(st[:], pg[:], AF.Copy, scale=0.125)
            else:
                nc.vector.tensor_scalar_mul(st[:], pg[:], 0.125)
            nc.gpsimd.tensor_tensor(
                et[:], ec[:].broadcast_to([P, 1024]), st[:], op=ALU.pow
            )
        else:
            nc.scalar.activation(et[:], pg[:], AF.Exp, scale=0.125)

    def attn_scores(p, sh, flush):
        # yields after each (h2, g) unit so pending-AV chains can interleave
        # into the PE queue (fills sc-rotation stall gaps; PE is in-order)
        qt, kt = QT[p], KT[p]
        exps = {}
        for g in range(NT // 2):
            for h2 in range(2):
                pg = psum.tile([P, 1024], F32, tag="sc", bufs=SC_BUFS, name="sc")
                for tt in range(2):
                    tau = 2 * g + tt
                    nc.tensor.matmul(
                        pg[:, 512 * tt : 512 * (tt + 1)],
                        kt[64 * h2 : 64 * (h2 + 1), P * tau : P * (tau + 1)],
                        qt[64 * h2 : 64 * (h2 + 1), SHW * sh : SHW * (sh + 1)],
                        start=True,
                        stop=True,
                    )
                et = expp.tile([P, 1024], F16, tag="exp", name="exp")
                emit_exp(et, pg)
                exps[(h2, g)] = et
                flush(1)
        return exps

    # ---- AV + post (output-natural orientation) ----
    OST = {}
    ost_done = {}

    def attn_post(p, sh, exps, sb):
        vs = VS[p]
        stile = 4 * sh + sb
        if stile not in OST:
            OST[stile] = opool.tile([P, D], F32, tag="ost", name=f"ost{stile}")
        ost = OST[stile]
        av = psum.tile([P, 512], F32, tag="sm", bufs=SM_BUFS, name="av")
        for h2 in range(2):
            for tau in range(NT):
                et = exps[(h2, tau // 2)]
                nc.tensor.matmul(
                    av[:, 65 * h2 : 65 * h2 + 65],
                    et[:, 512 * (tau % 2) + P * sb : 512 * (tau % 2) + P * (sb + 1)],
                    vs[:, 130 * tau + 65 * h2 : 130 * tau + 65 * h2 + 65],
                    start=(tau == 0),
                    stop=(tau == NT - 1),
                )
        a2 = av[:, 0:130].rearrange("p (h j) -> p h j", h=2, j=65)
        num = a2[:, :, 0:64]
        rc = rcp.tile([P, 2], F32, tag="rc", name="rc")
        nc.vector.reciprocal(rc[:], a2[:, :, 64])
        rcb = rc[:].unsqueeze(-1).broadcast_to([P, 2, 64])
        dst = ost[:, P * p : P * (p + 1)].rearrange(
            "p (h j) -> p h j", h=2, j=64
        )
        nc.vector.tensor_tensor(dst, num, rcb, op=ALU.mult)
        nc.sync.dma_start(
            out=out[P * stile : P * (stile + 1), P * p : P * (p + 1)],
            in_=ost[:, P * p : P * (p + 1)],
        )

    # ---- software-pipelined sweep ----
    emit_x()
    emit_qk(0)
    emit_qk(1)
    items = [(sh, p) for sh in range(2) for p in range(NPAIR)]
    # queue of pending AV-chain units (p, sh, exps, sb), drained into the
    # scores stream two units per unit emitted
    pend = []

    def drain():
        pp, psh, pexps = pend.pop(0)
        for sb in range(4):
            attn_post(pp, psh, pexps, sb)

    LAG = 3
    UPFRONT = 0
    if UPFRONT:
        for p in range(2, NPAIR):
            emit_qk(p)
        for p in range(NPAIR):
            emit_v(p)
    for i, (sh, p) in enumerate(items):
        exps = attn_scores(p, sh, lambda n: None)
        if not UPFRONT:
            if sh == 0:
                emit_v(p)
            if i + 2 < len(items) and items[i + 2][0] == 0:
                emit_qk(items[i + 2][1])
        pend.append((p, sh, exps))
        if len(pend) > LAG:
            drain()
    while pend:
        drain()


_NC_CACHE = {}


def build_nc(reps=1):
    if reps in _NC_CACHE:
        return _NC_CACHE[reps]
    nc = bacc.Bacc("TRN2", target_bir_lowering=False, debug=False)
    xT = nc.dram_tensor("xT", [D, S], F32R, kind="ExternalInput")
    xTb = nc.dram_tensor("xTb", [D, S], F16, kind="ExternalInput")
    wqk = nc.dram_tensor("wqk", [128, NPAIR * 256], F32R, kind="ExternalInput")
    wvb = nc.dram_tensor("wvb", [128, NPAIR * 128], F16, kind="ExternalInput")
    bqk = nc.dram_tensor("bqk", [128, 2 * NPAIR], F32, kind="ExternalInput")
    bvf = nc.dram_tensor("bvf", [128, D], F32, kind="ExternalInput")
    out = nc.dram_tensor("out", [S, D], F32, kind="ExternalOutput")
    from contextlib import ExitStack

    with tile.TileContext(nc) as tc:
        with ExitStack() as ctx:
            _emit(ctx, tc, nc, xT[:], xTb, wqk, wvb, bqk, bvf, out[:])
    nc.finalize()
    _NC_CACHE[reps] = nc
    return nc


def host_prep(sequences, Wq, bq, Wk, bk, Wv, bv):
    """Build the per-core input maps (host-side sharding + layout prep)."""
    sequences = np.asarray(sequences, np.float32)
    Wq, Wk, Wv = (np.asarray(a, np.float32) for a in (Wq, Wk, Wv))
    bq, bk, bv = (np.asarray(a, np.float32) for a in (bq, bk, bv))

    # block-diagonal head-pair Q/K weights, fp32r, K=128
    wqk = np.zeros((2 * NPAIR, 128, 128), np.float32)
    for p in range(NPAIR):
        for which, W in ((0, Wq), (1, Wk)):
            wqk[2 * p + which, 0:64, 0:64] = W[2 * p].T
            wqk[2 * p + which, 64:128, 64:128] = W[2 * p + 1].T
    wqk = np.ascontiguousarray(wqk.transpose(1, 0, 2)).reshape(128, 2 * NPAIR * 128)

    # block-diagonal V weights, fp16, [d(128) x (h0 64 | h1 64)] per pair
    wvb = np.zeros((NPAIR, 128, 128), np.float32)
    for p in range(NPAIR):
        wvb[p, 0:64, 0:64] = Wv[2 * p].T
        wvb[p, 64:128, 64:128] = Wv[2 * p + 1].T
    wvb = np.ascontiguousarray(wvb.transpose(1, 0, 2)).reshape(128, NPAIR * 128)

    bqk_t = np.zeros((128, 2 * NPAIR), np.float32)
    for p in range(NPAIR):
        bqk_t[0:64, 2 * p] = bq[2 * p]
        bqk_t[64:128, 2 * p] = bq[2 * p + 1]
        bqk_t[0:64, 2 * p + 1] = bk[2 * p]
        bqk_t[64:128, 2 * p + 1] = bk[2 * p + 1]
    bvf = np.tile(bv.reshape(1, D), (128, 1)).astype(np.float32)

    shared = {
        "wqk": wqk.astype(np.float32),
        "wvb": wvb.astype(np.float16),
        "bqk": bqk_t,
        "bvf": bvf,
    }
    in_maps = []
    for b in range(NCORES):
        xTb_full = np.ascontiguousarray(sequences[b].T)
        in_maps.append(
            {
                "xT": xTb_full.astype(np.float32),
                "xTb": xTb_full.astype(np.float16),
                **shared,
            }
        )
    return in_maps


def kernel(**inputs):
    nc = build_nc()
    in_maps = host_prep(
        inputs["sequences"],
        inputs["Wq"],
        inputs["bq"],
        inputs["Wk"],
        inputs["bk"],
        inputs["Wv"],
        inputs["bv"],
    )
    res = bass_utils.run_bass_kernel_spmd(
        nc, in_maps, core_ids=list(range(NCORES))
    )
    return np.stack([r["out"] for r in res.results], axis=0).astype(np.float32)
